# revision 20
# baseline (speedup 1.0000x reference)
"""BitLinear (int8-activation x ternary-weight) matmul on 8 TRN2 NeuronCores.

Full inputs: x [4, 4096, 2048] f32, weight [2048, 2048] f32.
Output: [4, 4096, 2048] fp16 = ((qx @ qw.T) / si / sw).astype(f16).

Strategy: data-parallel over the 16384 rows (2048 rows/core); the
weight is replicated. One full W scan (DMA-bound, ~50us) feeds exact
f32 DVE absolute-add reduces for mean|W|; NCACHE k-tiles stay parked
in SBUF and the rest are re-read as half-tiles issued behind the scan
on the same queue, self-paced by the quantizer. W quant is one DVE
magic-round (w*sw + 1.5*2^23) plus one ACT Sign(u - MAGIC) straight
to fp8 (sign(n) == clip(n,-1,1) for integer n). Per-row activation
quantization to int8 values held in bf16 runs entirely on the scalar
engine (scale+MAGIC then -MAGIC, both fp32-internal Copy activations)
so the vector engine only does the amax reduce - and the DMA-xbar
block transpose fires with zero cross-engine stall right after. The
matmul runs bf16(lhsT=qx^T) x fp8(qw^T) with fp32 PSUM accumulation
(exact), dequant fused into the PSUM->SBUF fp16 copy. The first two
row tiles interleave across k so each quantized k-tile unlocks 8
matmuls during the ramp; the ramp self-warms the PE clock gate and
the stream never gaps past the HAM idle window afterwards. Host only
reshapes/shards and transposes W (layout prep, no math).
"""

import numpy as np

import concourse.mybir as mybir
import concourse.tile as tile
from concourse import bacc
from concourse.bass import ts
from concourse.bass_utils import run_bass_kernel_spmd

N_CORES = 8
ROWS_TOTAL = 4 * 4096
K = 2048
N = 2048
NCACHE = 6  # W k-tiles kept in SBUF between the mean pass and quantize pass
MAGIC = 12582912.0  # 1.5*2^23: fp32 round-to-nearest-even (both signs)

f32 = mybir.dt.float32
bf16 = mybir.dt.bfloat16
f16 = mybir.dt.float16
fp8 = mybir.dt.float8e4
Alu = mybir.AluOpType
Act = mybir.ActivationFunctionType
AxX = mybir.AxisListType.X


def build(rows_per_core=ROWS_TOTAL // N_CORES):
    nc = bacc.Bacc(
        "TRN2", target_bir_lowering=False, debug=False, num_devices=N_CORES
    )
    x_ext = nc.declare_dram_parameter("x", [rows_per_core, K], f32, isOutput=False)
    wt_ext = nc.declare_dram_parameter("wt", [K, N], f32, isOutput=False)
    out_ext = nc.declare_dram_parameter(
        "out", [rows_per_core, N], f16, isOutput=True
    )

    KT = K // 128
    MT = rows_per_core // 128
    NQ = N // 512
    NPRE = min(3, MT)  # x tiles DMA-prefetched during the W scan

    with tile.TileContext(nc) as tc:
        with (
            tc.tile_pool(name="xin", bufs=3) as xin,  # [128,K] f32 x loads
            tc.tile_pool(name="wld", bufs=3) as wld,  # uncached W first reads
            tc.tile_pool(name="wch", bufs=NCACHE) as wch,  # parked W tiles
            tc.tile_pool(name="wre", bufs=6) as wre,  # [128,1024] re-read halves
            tc.tile_pool(name="scaled", bufs=2) as scaled,  # [128,K] f32 ACT out
            tc.tile_pool(name="qtmp", bufs=2) as qtmp,  # qx bf16
            tc.tile_pool(name="qxt", bufs=4) as qxtp,  # [128,KT,128] bf16 x^T
            tc.tile_pool(name="outp", bufs=3) as outp,  # [128,N] f16 results
            tc.tile_pool(name="singles", bufs=1) as singles,
            tc.tile_pool(name="small", bufs=8) as small,  # [128,1] stats
            tc.tile_pool(name="pacc", bufs=8, space="PSUM") as pacc,
        ):
            ones_f32 = singles.tile([128, 128], f32)
            nc.vector.memset(ones_f32, 1.0)
            negmagic_b = singles.tile([128, 1], f32)
            nc.vector.memset(negmagic_b, -MAGIC)
            qwT = singles.tile([128, KT, N], fp8)
            wsums = singles.tile([128, KT], f32)

            x_pre = {}
            w_tiles = {}
            xq = {}

            gates = {}

            def issue_x(mi):
                x_t = xin.tile([128, K], f32, tag="xin", name=f"x{mi}")
                if "late" in gates:
                    # keep steady-state x chains out of the quant window
                    nc.vector.tensor_copy(out=x_t[:, 0:1], in_=gates["late"])
                nc.sync.dma_start(out=x_t, in_=x_ext[ts(mi, 128), :])
                x_pre[mi] = x_t

            def x_amax(mi):
                # DVE part only: amax -> si (the heavy ops live on ACT)
                x_t = x_pre[mi]
                amax = small.tile([128, 1], f32, tag="small")
                nc.vector.tensor_reduce(
                    out=amax, in_=x_t, axis=AxX, op=Alu.max,
                    apply_absolute_value=True,
                )
                amc = small.tile([128, 1], f32, tag="amc", name=f"amc{mi}")
                nc.vector.tensor_scalar_max(out=amc, in0=amax, scalar1=1e-5)
                rec = small.tile([128, 1], f32, tag="small")
                nc.vector.reciprocal(out=rec, in_=amc)
                si = small.tile([128, 1], f32, tag="small")
                nc.vector.tensor_scalar_mul(out=si, in0=rec, scalar1=127.0)
                return amc, si

            def x_finish_quant(mi, amc, si, gate=None):
                # Pass 1: u = x*si + MAGIC (fp32-internal; the +MAGIC
                # rounds x*si to the nearest integer, RNE). Pass 2:
                # qx = u - MAGIC emitted as bf16, then the block
                # transpose. For the prequant tiles pass 2 subtracts a
                # post-sw [-MAGIC] tile instead of an immediate: the
                # strided transpose DMA then cannot be scheduled into the
                # W scan, whose HBM stream it would hole-punch (~7us per
                # transpose measured).
                x_t = x_pre.pop(mi)
                xs = scaled.tile([128, K], f32, tag="scaled")
                nc.scalar.activation(
                    out=xs, in_=x_t, func=Act.Copy, scale=si, bias=MAGIC
                )
                qx = qtmp.tile([128, K], bf16, tag="qtmp")
                if gate is None:
                    nc.scalar.activation(
                        out=qx, in_=xs, func=Act.Copy, bias=-MAGIC
                    )
                else:
                    nc.vector.tensor_scalar_add(out=qx, in0=xs, scalar1=gate)
                qxT = qxtp.tile(
                    [128, KT, 128], bf16, tag="qxt", name=f"qxT{mi}"
                )
                nc.scalar.dma_start_transpose(out=qxT, in_=qx)
                xq[mi] = (qxT, amc)

            def x_quant(mi):
                if mi not in xq:
                    if mi not in x_pre:
                        issue_x(mi)
                    amc, si = x_amax(mi)
                    x_finish_quant(mi, amc, si)
                return xq.pop(mi)

            # ---- W scan: one DMA pass feeds the exact f32 mean reduces;
            # x0-x2 ride along (x0/x1 quantized inline, x2 deferred)
            for kt in range(KT):
                if kt < NCACHE:
                    wt_t = wch.tile([128, K], f32, tag="wch", name=f"wch{kt}")
                else:
                    wt_t = wld.tile([128, K], f32, tag="wld", name=f"wf{kt}")
                nc.sync.dma_start(out=wt_t, in_=wt_ext[ts(kt, 128), :])
                w_tiles[kt] = wt_t
                nc.vector.tensor_reduce(
                    out=wsums[:, kt : kt + 1], in_=wt_t, axis=AxX,
                    op=Alu.add, apply_absolute_value=True,
                )
                if kt == 0:
                    issue_x(0)
                    pre_si = {0: x_amax(0)}
                elif kt == 5:
                    issue_x(1)
                    pre_si[1] = x_amax(1)

            # ---- sw = 1/max(mean|W|, 1e-5)
            wtot = small.tile([128, 1], f32, tag="s1")
            nc.vector.tensor_reduce(out=wtot, in_=wsums, axis=AxX, op=Alu.add)
            # ones^T @ wtot replicates the grand total across partitions
            ptot_b = pacc.tile([128, 1], f32, tag="acc", name="ptot_b")
            nc.tensor.matmul(ptot_b, lhsT=ones_f32, rhs=wtot, start=True, stop=True)
            meanc_b = small.tile([128, 1], f32, tag="s4")
            nc.vector.tensor_scalar(
                out=meanc_b, in0=ptot_b, scalar1=1.0 / (K * N), scalar2=1e-5,
                op0=Alu.mult, op1=Alu.max,
            )
            sw_b = singles.tile([128, 1], f32)
            nc.vector.reciprocal(out=sw_b, in_=meanc_b)
            q_b = singles.tile([128, 1], f32)
            nc.vector.tensor_scalar_mul(out=q_b, in0=meanc_b, scalar1=1.0 / 127.0)
            # scheduling gates, ready the moment the mean scan drains:
            # pre-touching a tile with one forces everything downstream
            # of it (DMA re-reads, transposes, late x chains) out of the
            # scan's HBM stream, which they would otherwise hole-punch
            gate_one = singles.tile([128, 1], f32)
            nc.vector.tensor_scalar(
                out=gate_one, in0=wtot, scalar1=0.0, scalar2=1.0,
                op0=Alu.mult, op1=Alu.add,
            )
            negmagic_gate = singles.tile([128, 1], f32)
            nc.vector.tensor_scalar(
                out=negmagic_gate, in0=wtot, scalar1=0.0, scalar2=-MAGIC,
                op0=Alu.mult, op1=Alu.add,
            )
            for mi in sorted(pre_si):
                x_finish_quant(mi, *pre_si[mi], gate=negmagic_gate)

            # re-reads for the non-cached k-tiles, gated behind the scan
            wre_tiles = {}
            for kt in range(NCACHE, KT):
                for h in range(2):
                    r_t = wre.tile([128, 1024], f32, tag="wre", name=f"wr{kt}_{h}")
                    nc.vector.tensor_copy(out=r_t[:, 0:1], in_=gate_one)
                    nc.sync.dma_start(
                        out=r_t, in_=wt_ext[ts(kt, 128), ts(h, 1024)]
                    )
                    wre_tiles[(kt, h)] = r_t

            # x2: DMA + full chain post-sw (its amax would otherwise be
            # interleaved into the scan reduces / quant chain by the
            # scheduler); needed only when tile 2 starts, after the ramp
            if NPRE > 2:
                x2_t = xin.tile([128, K], f32, tag="xin", name="x2")
                nc.vector.tensor_copy(out=x2_t[:, 0:1], in_=gate_one)
                nc.sync.dma_start(out=x2_t, in_=x_ext[ts(2, 128), :])
                x_pre[2] = x2_t
                pre2 = x_amax(2)
                x_finish_quant(2, *pre2, gate=negmagic_gate)

            # ---- W quant: qwT = sign(round(wT*sw)) as fp8, parked tiles
            # first so the re-read halves have time to arrive
            def quant_w(src, kt, n0, width):
                nc.vector.tensor_scalar(
                    out=src, in0=src, scalar1=sw_b, scalar2=MAGIC,
                    op0=Alu.mult, op1=Alu.add,
                )
                nc.scalar.activation(
                    out=qwT[:, kt, n0 : n0 + width], in_=src,
                    func=Act.Sign, bias=negmagic_b,
                )

            for kt in range(NCACHE):
                quant_w(w_tiles[kt], kt, 0, K)
            for kt in range(NCACHE, KT):
                for h in range(2):
                    quant_w(wre_tiles[(kt, h)], kt, h * 1024, 1024)

            # gate for the steady-state x pipeline: ready once the last
            # k-tile is quantized, so x3+'s DMA/amax/scale/transpose work
            # cannot be scheduled into the ramp's critical quant chain
            qw_gate = singles.tile([128, 1], f32)
            nc.vector.tensor_scalar(
                out=qw_gate, in0=qwT[:, KT - 1, 0:1], scalar1=0.0, scalar2=1.0,
                op0=Alu.mult, op1=Alu.add,
            )
            gates["late"] = qw_gate

            # ---- main loop over row tiles
            def mm(acc, qxT, kt, nq):
                nc.tensor.matmul(
                    acc, lhsT=qxT[:, kt, :], rhs=qwT[:, kt, ts(nq, 512)],
                    start=(kt == 0), stop=(kt == KT - 1),
                    skip_group_check=True,
                )

            def finish(mi, accs, amc):
                cs = small.tile([128, 1], f32, tag="small")
                nc.vector.tensor_mul(cs, amc, q_b)  # (amax/127)*meanc
                o_t = outp.tile([128, N], f16, tag="outp", name=f"o{mi}")
                for nq in range(NQ):
                    nc.scalar.activation(
                        out=o_t[:, ts(nq, 512)], in_=accs[nq],
                        func=Act.Copy, scale=cs,
                    )
                nc.scalar.dma_start(out=out_ext[ts(mi, 128), :], in_=o_t)

            if MT >= 2:
                # interleave the first two row tiles across kt so each
                # quantized qwT k-tile unlocks 8 matmuls during the ramp
                qxT0, amc0 = x_quant(0)
                qxT1, amc1 = x_quant(1)
                accs0 = [
                    pacc.tile([128, 512], f32, tag="acc", name=f"acc_0_{i}")
                    for i in range(NQ)
                ]
                accs1 = [
                    pacc.tile([128, 512], f32, tag="acc", name=f"acc_1_{i}")
                    for i in range(NQ)
                ]
                for kt in range(KT):
                    for nq in range(NQ):
                        mm(accs0[nq], qxT0, kt, nq)
                    for nq in range(NQ):
                        mm(accs1[nq], qxT1, kt, nq)
                finish(0, accs0, amc0)
                finish(1, accs1, amc1)
                start_mi = 2
            else:
                start_mi = 0

            for mi in range(start_mi, MT):
                qxT, amc = x_quant(mi)
                accs = [
                    pacc.tile([128, 512], f32, tag="acc", name=f"acc_{mi}_{i}")
                    for i in range(NQ)
                ]
                if mi == MT - 1:
                    # nq-inner: each output chunk completes as soon as its
                    # 16 accumulations are done, so the dequant + store
                    # overlap the remaining matmuls (shorter kernel tail)
                    for nq in range(NQ):
                        for kt in range(KT):
                            mm(accs[nq], qxT, kt, nq)
                else:
                    for kt in range(KT):
                        for nq in range(NQ):
                            mm(accs[nq], qxT, kt, nq)
                finish(mi, accs, amc)

    nc.compile()
    return nc


_NC_CACHE = {}


def _get_nc(rows_per_core):
    if rows_per_core not in _NC_CACHE:
        _NC_CACHE[rows_per_core] = build(rows_per_core)
    return _NC_CACHE[rows_per_core]


def run(x, weight, **spmd_kwargs):
    x = np.ascontiguousarray(np.asarray(x, dtype=np.float32))
    weight = np.asarray(weight, dtype=np.float32)
    b, s, k = x.shape
    rows = b * s
    rpc = rows // N_CORES
    xr = x.reshape(rows, k)
    wt = np.ascontiguousarray(weight.T)
    nc = _get_nc(rpc)
    in_maps = [
        {"x": xr[i * rpc : (i + 1) * rpc], "wt": wt} for i in range(N_CORES)
    ]
    res = run_bass_kernel_spmd(
        nc, in_maps, core_ids=list(range(N_CORES)), **spmd_kwargs
    )
    out = np.concatenate(
        [res.results[i]["out"] for i in range(N_CORES)], axis=0
    )
    return out.reshape(b, s, N), res


def kernel(x, weight):
    out, _ = run(x, weight)
    return out


# revision 21
# speedup vs baseline: 1.0542x; 1.0542x over previous
"""BitLinear (int8-activation x ternary-weight) matmul on 8 TRN2 NeuronCores.

Full inputs: x [4, 4096, 2048] f32, weight [2048, 2048] f32.
Output: [4, 4096, 2048] fp16 = ((qx @ qw.T) / si / sw).astype(f16).

Strategy: data-parallel over the 16384 rows (2048 rows/core); the
weight is replicated. One full W scan (DMA-bound, ~50us) feeds exact
f32 DVE absolute-add reduces for mean|W|; NCACHE k-tiles stay parked
in SBUF and the rest are re-read as half-tiles issued behind the scan
on the same queue, self-paced by the quantizer. W quant is one DVE
magic-round (w*sw + 1.5*2^23) plus one ACT Sign(u - MAGIC) straight
to fp8 (sign(n) == clip(n,-1,1) for integer n). Per-row activation
quantization to int8 values held in bf16 runs entirely on the scalar
engine (scale+MAGIC then -MAGIC, both fp32-internal Copy activations)
so the vector engine only does the amax reduce - and the DMA-xbar
block transpose fires with zero cross-engine stall right after. The
matmul runs bf16(lhsT=qx^T) x fp8(qw^T) with fp32 PSUM accumulation
(exact), dequant fused into the PSUM->SBUF fp16 copy. The first two
row tiles interleave across k so each quantized k-tile unlocks 8
matmuls during the ramp; the ramp self-warms the PE clock gate and
the stream never gaps past the HAM idle window afterwards. Host only
reshapes/shards and transposes W (layout prep, no math).
"""

import numpy as np

import concourse.mybir as mybir
import concourse.tile as tile
from concourse import bacc
from concourse.bass import ts
from concourse.bass_utils import run_bass_kernel_spmd

N_CORES = 8
ROWS_TOTAL = 4 * 4096
K = 2048
N = 2048
NCACHE = 6  # W k-tiles kept in SBUF between the mean pass and quantize pass
MAGIC = 12582912.0  # 1.5*2^23: fp32 round-to-nearest-even (both signs)

f32 = mybir.dt.float32
bf16 = mybir.dt.bfloat16
f16 = mybir.dt.float16
fp8 = mybir.dt.float8e4
Alu = mybir.AluOpType
Act = mybir.ActivationFunctionType
AxX = mybir.AxisListType.X


def build(rows_per_core=ROWS_TOTAL // N_CORES):
    nc = bacc.Bacc(
        "TRN2", target_bir_lowering=False, debug=False, num_devices=N_CORES
    )
    x_ext = nc.declare_dram_parameter("x", [rows_per_core, K], f32, isOutput=False)
    wt_ext = nc.declare_dram_parameter("wt", [K, N], f32, isOutput=False)
    out_ext = nc.declare_dram_parameter(
        "out", [rows_per_core, N], f16, isOutput=True
    )

    KT = K // 128
    MT = rows_per_core // 128
    NQ = N // 512
    NPRE = min(3, MT)  # x tiles DMA-prefetched during the W scan

    with tile.TileContext(nc) as tc:
        with (
            tc.tile_pool(name="xin", bufs=3) as xin,  # [128,K] f32 x loads
            tc.tile_pool(name="wld", bufs=3) as wld,  # uncached W first reads
            tc.tile_pool(name="wch", bufs=NCACHE) as wch,  # parked W tiles
            tc.tile_pool(name="wre", bufs=6) as wre,  # [128,1024] re-read halves
            tc.tile_pool(name="scaled", bufs=2) as scaled,  # [128,K] f32 ACT out
            tc.tile_pool(name="qtmp", bufs=2) as qtmp,  # qx bf16
            tc.tile_pool(name="qxt", bufs=4) as qxtp,  # [128,KT,128] bf16 x^T
            tc.tile_pool(name="outp", bufs=3) as outp,  # [128,N] f16 results
            tc.tile_pool(name="singles", bufs=1) as singles,
            tc.tile_pool(name="small", bufs=8) as small,  # [128,1] stats
            tc.tile_pool(name="pacc", bufs=8, space="PSUM") as pacc,
        ):
            ones_f32 = singles.tile([128, 128], f32)
            nc.vector.memset(ones_f32, 1.0)
            negmagic_b = singles.tile([128, 1], f32)
            nc.vector.memset(negmagic_b, -MAGIC)
            qwT = singles.tile([128, KT, N], fp8)
            wsums = singles.tile([128, KT], f32)

            x_pre = {}
            w_tiles = {}
            xq = {}

            gates = {}

            def issue_x(mi):
                x_t = xin.tile([128, K], f32, tag="xin", name=f"x{mi}")
                if "late" in gates:
                    # keep steady-state x chains out of the quant window
                    nc.vector.tensor_copy(out=x_t[:, 0:1], in_=gates["late"])
                nc.sync.dma_start(out=x_t, in_=x_ext[ts(mi, 128), :])
                x_pre[mi] = x_t

            def x_amax(mi):
                # DVE part only: amax -> si (the heavy ops live on ACT)
                x_t = x_pre[mi]
                amax = small.tile([128, 1], f32, tag="small")
                nc.vector.tensor_reduce(
                    out=amax, in_=x_t, axis=AxX, op=Alu.max,
                    apply_absolute_value=True,
                )
                amc = small.tile([128, 1], f32, tag="amc", name=f"amc{mi}")
                nc.vector.tensor_scalar_max(out=amc, in0=amax, scalar1=1e-5)
                rec = small.tile([128, 1], f32, tag="small")
                nc.vector.reciprocal(out=rec, in_=amc)
                si = small.tile([128, 1], f32, tag="small")
                nc.vector.tensor_scalar_mul(out=si, in0=rec, scalar1=127.0)
                return amc, si

            def x_finish_quant(mi, amc, si, gate=None):
                # Pass 1: u = x*si + MAGIC (fp32-internal; the +MAGIC
                # rounds x*si to the nearest integer, RNE). Pass 2:
                # qx = u - MAGIC emitted as bf16, then the block
                # transpose. For the prequant tiles pass 2 subtracts a
                # post-sw [-MAGIC] tile instead of an immediate: the
                # strided transpose DMA then cannot be scheduled into the
                # W scan, whose HBM stream it would hole-punch (~7us per
                # transpose measured).
                x_t = x_pre.pop(mi)
                xs = scaled.tile([128, K], f32, tag="scaled")
                nc.scalar.activation(
                    out=xs, in_=x_t, func=Act.Copy, scale=si, bias=MAGIC
                )
                qx = qtmp.tile([128, K], bf16, tag="qtmp")
                if gate is None:
                    nc.scalar.activation(
                        out=qx, in_=xs, func=Act.Copy, bias=-MAGIC
                    )
                else:
                    nc.vector.tensor_scalar_add(out=qx, in0=xs, scalar1=gate)
                qxT = qxtp.tile(
                    [128, KT, 128], bf16, tag="qxt", name=f"qxT{mi}"
                )
                nc.scalar.dma_start_transpose(out=qxT, in_=qx)
                xq[mi] = (qxT, amc)

            def x_quant(mi):
                if mi not in xq:
                    if mi not in x_pre:
                        issue_x(mi)
                    amc, si = x_amax(mi)
                    x_finish_quant(mi, amc, si)
                return xq.pop(mi)

            # ---- W scan: one DMA pass feeds the exact f32 mean reduces;
            # x0-x2 ride along (x0/x1 quantized inline, x2 deferred)
            for kt in range(KT):
                if kt < NCACHE:
                    wt_t = wch.tile([128, K], f32, tag="wch", name=f"wch{kt}")
                else:
                    wt_t = wld.tile([128, K], f32, tag="wld", name=f"wf{kt}")
                nc.sync.dma_start(out=wt_t, in_=wt_ext[ts(kt, 128), :])
                w_tiles[kt] = wt_t
                nc.vector.tensor_reduce(
                    out=wsums[:, kt : kt + 1], in_=wt_t, axis=AxX,
                    op=Alu.add, apply_absolute_value=True,
                )
                if kt == 0:
                    issue_x(0)
                    pre_si = {0: x_amax(0)}
                elif kt == 5:
                    issue_x(1)
                    pre_si[1] = x_amax(1)

            # ---- sw = 1/max(mean|W|, 1e-5)
            wtot = small.tile([128, 1], f32, tag="s1")
            nc.vector.tensor_reduce(out=wtot, in_=wsums, axis=AxX, op=Alu.add)
            # ones^T @ wtot replicates the grand total across partitions
            ptot_b = pacc.tile([128, 1], f32, tag="acc", name="ptot_b")
            nc.tensor.matmul(ptot_b, lhsT=ones_f32, rhs=wtot, start=True, stop=True)
            meanc_b = small.tile([128, 1], f32, tag="s4")
            nc.vector.tensor_scalar(
                out=meanc_b, in0=ptot_b, scalar1=1.0 / (K * N), scalar2=1e-5,
                op0=Alu.mult, op1=Alu.max,
            )
            sw_b = singles.tile([128, 1], f32)
            nc.vector.reciprocal(out=sw_b, in_=meanc_b)
            q_b = singles.tile([128, 1], f32)
            nc.vector.tensor_scalar_mul(out=q_b, in0=meanc_b, scalar1=1.0 / 127.0)
            # scheduling gates, ready the moment the mean scan drains:
            # pre-touching a tile with one forces everything downstream
            # of it (DMA re-reads, transposes, late x chains) out of the
            # scan's HBM stream, which they would otherwise hole-punch
            gate_one = singles.tile([128, 1], f32)
            nc.vector.tensor_scalar(
                out=gate_one, in0=wtot, scalar1=0.0, scalar2=1.0,
                op0=Alu.mult, op1=Alu.add,
            )
            negmagic_gate = singles.tile([128, 1], f32)
            nc.vector.tensor_scalar(
                out=negmagic_gate, in0=wtot, scalar1=0.0, scalar2=-MAGIC,
                op0=Alu.mult, op1=Alu.add,
            )
            for mi in sorted(pre_si):
                x_finish_quant(mi, *pre_si[mi], gate=negmagic_gate)

            # re-reads for the non-cached k-tiles, gated behind the scan
            wre_tiles = {}
            for kt in range(NCACHE, KT):
                for h in range(2):
                    r_t = wre.tile([128, 1024], f32, tag="wre", name=f"wr{kt}_{h}")
                    nc.vector.tensor_copy(out=r_t[:, 0:1], in_=gate_one)
                    nc.sync.dma_start(
                        out=r_t, in_=wt_ext[ts(kt, 128), ts(h, 1024)]
                    )
                    wre_tiles[(kt, h)] = r_t

            # x2: DMA + full chain post-sw (its amax would otherwise be
            # interleaved into the scan reduces / quant chain by the
            # scheduler); needed only when tile 2 starts, after the ramp
            if NPRE > 2:
                x2_t = xin.tile([128, K], f32, tag="xin", name="x2")
                nc.vector.tensor_copy(out=x2_t[:, 0:1], in_=gate_one)
                nc.sync.dma_start(out=x2_t, in_=x_ext[ts(2, 128), :])
                x_pre[2] = x2_t
                pre2 = x_amax(2)
                x_finish_quant(2, *pre2, gate=negmagic_gate)

            # ---- W quant: qwT = sign(round(wT*sw)) as fp8, parked tiles
            # first so the re-read halves have time to arrive
            def quant_w(src, kt, n0, width):
                nc.vector.tensor_scalar(
                    out=src, in0=src, scalar1=sw_b, scalar2=MAGIC,
                    op0=Alu.mult, op1=Alu.add,
                )
                nc.scalar.activation(
                    out=qwT[:, kt, n0 : n0 + width], in_=src,
                    func=Act.Sign, bias=negmagic_b,
                )

            for kt in range(NCACHE):
                quant_w(w_tiles[kt], kt, 0, K)
            for kt in range(NCACHE, KT):
                for h in range(2):
                    quant_w(wre_tiles[(kt, h)], kt, h * 1024, 1024)

            # x3+ stays ungated: the scheduler's free-running lookahead on
            # the steady x chains is what keeps the stream gapless (v6
            # experiment: gating them on the last sign bunched 13 tiles'
            # chains post-quant and starved the early stream)

            # ---- main loop over row tiles
            def mm(acc, qxT, kt, nq):
                nc.tensor.matmul(
                    acc, lhsT=qxT[:, kt, :], rhs=qwT[:, kt, ts(nq, 512)],
                    start=(kt == 0), stop=(kt == KT - 1),
                    skip_group_check=True,
                )

            def finish(mi, accs, amc):
                cs = small.tile([128, 1], f32, tag="small")
                nc.vector.tensor_mul(cs, amc, q_b)  # (amax/127)*meanc
                o_t = outp.tile([128, N], f16, tag="outp", name=f"o{mi}")
                for nq in range(NQ):
                    nc.scalar.activation(
                        out=o_t[:, ts(nq, 512)], in_=accs[nq],
                        func=Act.Copy, scale=cs,
                    )
                nc.scalar.dma_start(out=out_ext[ts(mi, 128), :], in_=o_t)

            if MT >= 2:
                # interleave the first two row tiles across kt so each
                # quantized qwT k-tile unlocks 8 matmuls during the ramp
                qxT0, amc0 = x_quant(0)
                qxT1, amc1 = x_quant(1)
                accs0 = [
                    pacc.tile([128, 512], f32, tag="acc", name=f"acc_0_{i}")
                    for i in range(NQ)
                ]
                accs1 = [
                    pacc.tile([128, 512], f32, tag="acc", name=f"acc_1_{i}")
                    for i in range(NQ)
                ]
                for kt in range(KT):
                    for nq in range(NQ):
                        mm(accs0[nq], qxT0, kt, nq)
                    for nq in range(NQ):
                        mm(accs1[nq], qxT1, kt, nq)
                finish(0, accs0, amc0)
                finish(1, accs1, amc1)
                start_mi = 2
            else:
                start_mi = 0

            for mi in range(start_mi, MT):
                qxT, amc = x_quant(mi)
                accs = [
                    pacc.tile([128, 512], f32, tag="acc", name=f"acc_{mi}_{i}")
                    for i in range(NQ)
                ]
                if mi == MT - 1:
                    # nq-inner: each output chunk completes as soon as its
                    # 16 accumulations are done, so the dequant + store
                    # overlap the remaining matmuls (shorter kernel tail)
                    for nq in range(NQ):
                        for kt in range(KT):
                            mm(accs[nq], qxT, kt, nq)
                else:
                    for kt in range(KT):
                        for nq in range(NQ):
                            mm(accs[nq], qxT, kt, nq)
                finish(mi, accs, amc)

    nc.compile()
    return nc


_NC_CACHE = {}


def _get_nc(rows_per_core):
    if rows_per_core not in _NC_CACHE:
        _NC_CACHE[rows_per_core] = build(rows_per_core)
    return _NC_CACHE[rows_per_core]


def run(x, weight, **spmd_kwargs):
    x = np.ascontiguousarray(np.asarray(x, dtype=np.float32))
    weight = np.asarray(weight, dtype=np.float32)
    b, s, k = x.shape
    rows = b * s
    rpc = rows // N_CORES
    xr = x.reshape(rows, k)
    wt = np.ascontiguousarray(weight.T)
    nc = _get_nc(rpc)
    in_maps = [
        {"x": xr[i * rpc : (i + 1) * rpc], "wt": wt} for i in range(N_CORES)
    ]
    res = run_bass_kernel_spmd(
        nc, in_maps, core_ids=list(range(N_CORES)), **spmd_kwargs
    )
    out = np.concatenate(
        [res.results[i]["out"] for i in range(N_CORES)], axis=0
    )
    return out.reshape(b, s, N), res


def kernel(x, weight):
    out, _ = run(x, weight)
    return out


# revision 25
# speedup vs baseline: 1.1760x; 1.1155x over previous
"""BitLinear (int8-activation x ternary-weight) matmul on 8 TRN2 NeuronCores.

Full inputs: x [4, 4096, 2048] f32, weight [2048, 2048] f32.
Output: [4, 4096, 2048] fp16 = ((qx @ qw.T) / si / sw).astype(f16).

Strategy: data-parallel over the 16384 rows (2048 rows/core). The weight
is replicated; each core computes mean|W| on-device (first W read),
then quantizes W to ternary {-1,0,1} stored as fp8 (cached k-tiles in
SBUF avoid most of the second read). Per-row activation quantization to
int8 values held in bf16 uses the fp32 magic-number trick
(v + 1.5*2^23 rounds to the nearest integer, RNE) and a DMA-xbar
block transpose. The matmul runs bf16(lhsT=qx^T) x fp8(qw^T) on the
TensorEngine with fp32 PSUM accumulation -- exact for these integer
values -- and the dequant (acc * amax/127 * mean|W|) is fused into the
PSUM->SBUF fp16 copy on the ScalarEngine. The first two row tiles are
interleaved across k so each arriving quantized W k-tile unlocks 8
matmuls during the W-prep ramp, and junk matmuls keep the PE's HAM
clock gate warm while the W mean pass runs. Host only reshapes/shards
and transposes W (layout prep, no math).
"""

import numpy as np

import concourse.mybir as mybir
import concourse.tile as tile
from concourse import bacc
from concourse.bass import ts
from concourse.bass_utils import run_bass_kernel_spmd

N_CORES = 8
ROWS_TOTAL = 4 * 4096
K = 2048
N = 2048
NCACHE = 8  # W k-tiles kept in SBUF between the mean pass and quantize pass
MAGIC = 12582912.0  # 1.5*2^23: fp32 round-to-nearest-even (both signs)

f32 = mybir.dt.float32
bf16 = mybir.dt.bfloat16
f16 = mybir.dt.float16
fp8 = mybir.dt.float8e4
Alu = mybir.AluOpType
Act = mybir.ActivationFunctionType
AxX = mybir.AxisListType.X


def build(rows_per_core=ROWS_TOTAL // N_CORES):
    nc = bacc.Bacc(
        "TRN2", target_bir_lowering=False, debug=False, num_devices=N_CORES
    )
    x_ext = nc.declare_dram_parameter("x", [rows_per_core, K], f32, isOutput=False)
    wt_ext = nc.declare_dram_parameter("wt", [K, N], f32, isOutput=False)
    out_ext = nc.declare_dram_parameter(
        "out", [rows_per_core, N], f16, isOutput=True
    )

    KT = K // 128
    MT = rows_per_core // 128
    NQ = N // 512
    NPRE = min(4, MT)  # x tiles prefetched during W prep

    with tile.TileContext(nc) as tc:
        with (
            tc.tile_pool(name="xin", bufs=3) as xin,  # [128,K] f32 x loads
            tc.tile_pool(name="wld", bufs=4) as wld,  # [128,K] f32 W loads
            tc.tile_pool(name="wch", bufs=NCACHE) as wch,  # cached W tiles
            tc.tile_pool(name="scaled", bufs=2) as scaled,  # [128,K] f32 ACT out
            tc.tile_pool(name="qtmp", bufs=3) as qtmp,  # rounded f32 / qx bf16
            tc.tile_pool(name="qxt", bufs=3) as qxtp,  # [128,KT,128] bf16 x^T
            tc.tile_pool(name="outp", bufs=3) as outp,  # [128,N] f16 results
            tc.tile_pool(name="singles", bufs=1) as singles,
            tc.tile_pool(name="small", bufs=6) as small,  # [128,1] stats
            tc.tile_pool(name="pacc", bufs=8, space="PSUM") as pacc,
        ):
            ones_mat = singles.tile([128, 128], f32)
            nc.vector.memset(ones_mat, 1.0)
            negmagic_b = singles.tile([128, 1], f32)
            nc.vector.memset(negmagic_b, -MAGIC)
            qwT = singles.tile([128, KT, N], fp8)
            wsums = singles.tile([128, KT], f32)

            def x_quant(mi):
                if mi in x_pre:
                    x_t = x_pre[mi]
                else:
                    x_t = xin.tile([128, K], f32, tag="xin", name=f"x{mi}")
                    nc.sync.dma_start(out=x_t, in_=x_ext[ts(mi, 128), :])
                amax = small.tile([128, 1], f32, tag="small")
                nc.vector.tensor_reduce(
                    out=amax, in_=x_t, axis=AxX, op=Alu.max,
                    apply_absolute_value=True,
                )
                amc = small.tile([128, 1], f32, tag="amc", name=f"amc{mi}")
                nc.vector.tensor_scalar_max(out=amc, in0=amax, scalar1=1e-5)
                rec = small.tile([128, 1], f32, tag="small")
                nc.vector.reciprocal(out=rec, in_=amc)
                si = small.tile([128, 1], f32, tag="small")
                nc.vector.tensor_scalar_mul(out=si, in0=rec, scalar1=127.0)
                # both passes on ACT (fp32-internal): pass 1's +MAGIC bias
                # rounds x*si to the nearest integer (RNE), pass 2 removes
                # it into bf16 -- the DVE only does the amax chain
                xs = scaled.tile([128, K], f32, tag="scaled")
                nc.scalar.activation(
                    out=xs, in_=x_t, func=Act.Copy, scale=si, bias=MAGIC
                )
                qx = qtmp.tile([128, K], bf16, tag="qtmp")
                nc.scalar.activation(out=qx, in_=xs, func=Act.Copy, bias=-MAGIC)
                qxT = qxtp.tile(
                    [128, KT, 128], bf16, tag="qxt", name=f"qxT{mi}"
                )
                nc.sync.dma_start_transpose(out=qxT, in_=qx)
                return qxT, amc

            # ---- PE warm-up: the HAM clock gate halves the PE clock after
            # ~3.4us idle, and the PE has no real work until quantized W
            # tiles arrive (~55us). Junk matmuls rotating through the 8
            # pacc slots (so Tile's same-bank serialization stays off the
            # back-to-back path) hold the clock at 2.4 GHz through the
            # W-prep head; the ramp matmuls then keep it warm.
            warm_src = singles.tile([128, 512], bf16)
            nc.vector.memset(warm_src, 1.0)
            for wi in range(120):
                pwarm = pacc.tile([128, 512], f32, tag="acc", name=f"warm{wi}")
                nc.tensor.matmul(
                    pwarm, lhsT=warm_src[:, :128], rhs=warm_src,
                    start=True, stop=True, skip_group_check=True,
                )

            # ---- W pass 1: mean(|W|); cache k-tiles 0..NCACHE-1 in SBUF
            wcache_tiles = {}
            for kt in range(KT):
                if kt < NCACHE:
                    wt_t = wch.tile([128, K], f32, tag="wch", name=f"wch{kt}")
                    wcache_tiles[kt] = wt_t
                else:
                    wt_t = wld.tile([128, K], f32, tag="wld", name=f"wld{kt}")
                nc.sync.dma_start(out=wt_t, in_=wt_ext[ts(kt, 128), :])
                nc.vector.tensor_reduce(
                    out=wsums[:, kt : kt + 1],
                    in_=wt_t,
                    axis=AxX,
                    op=Alu.add,
                    apply_absolute_value=True,
                )
            # first two x tiles next in the input FIFO: needed for the ramp
            x_pre = {}
            for mi in range(min(2, MT)):
                x_t = xin.tile([128, K], f32, tag="xin", name=f"xpre{mi}")
                nc.sync.dma_start(out=x_t, in_=x_ext[ts(mi, 128), :])
                x_pre[mi] = x_t
            # re-read DMAs for non-cached tiles: emitted before the x_quant
            # calls so they are not head-of-line blocked on the sync ring by
            # the transpose-issues (which wait on the DVE-produced qx tiles)
            wreread_tiles = {}
            for kt in range(NCACHE, KT):
                wt_t = wld.tile([128, K], f32, tag="wld", name=f"wldr{kt}")
                nc.sync.dma_start(out=wt_t, in_=wt_ext[ts(kt, 128), :])
                wreread_tiles[kt] = wt_t
            if MT >= 2:
                xq0 = x_quant(0)
                xq1 = x_quant(1)
            # then the next prefetched x tiles
            for mi in range(2, NPRE):
                x_t = xin.tile([128, K], f32, tag="xin", name=f"xpre{mi}")
                nc.sync.dma_start(out=x_t, in_=x_ext[ts(mi, 128), :])
                x_pre[mi] = x_t
            wtot = small.tile([128, 1], f32, tag="small")
            nc.vector.tensor_reduce(out=wtot, in_=wsums, axis=AxX, op=Alu.add)
            # ones_mat.T @ wtot replicates the grand total across all 128
            # partitions in one matmul, so the scale math runs as [128,1]
            # vectors with no extra broadcast round-trips
            ptot_b = pacc.tile([128, 1], f32, tag="acc", name="ptot_b")
            nc.tensor.matmul(ptot_b, lhsT=ones_mat, rhs=wtot, start=True, stop=True)
            # meanc = max(mean|W|, 1e-5); sw = 1/meanc; q = meanc/127
            meanc_b = small.tile([128, 1], f32, tag="s1")
            nc.vector.tensor_scalar(
                out=meanc_b,
                in0=ptot_b,
                scalar1=1.0 / (K * N),
                scalar2=1e-5,
                op0=Alu.mult,
                op1=Alu.max,
            )
            sw_b = singles.tile([128, 1], f32)
            nc.vector.reciprocal(out=sw_b, in_=meanc_b)
            q_b = singles.tile([128, 1], f32)
            nc.vector.tensor_scalar_mul(out=q_b, in0=meanc_b, scalar1=1.0 / 127.0)

            # ---- W pass 2: qwT = clip(round(wT*sw), -1, 1) as fp8
            # Two passes, one per engine: DVE computes u = w*sw + MAGIC
            # in-place (the fp32 add rounds to the nearest integer, RNE);
            # ACT then emits Sign(u - MAGIC) straight to fp8 -- for integer
            # n, sign(n) == clip(n, -1, 1).
            for kt in range(KT):
                wt_t = wcache_tiles.get(kt) or wreread_tiles.get(kt)
                nc.vector.tensor_scalar(
                    out=wt_t, in0=wt_t, scalar1=sw_b, scalar2=MAGIC,
                    op0=Alu.mult, op1=Alu.add,
                )
                nc.scalar.activation(
                    out=qwT[:, kt, :], in_=wt_t, func=Act.Sign, bias=negmagic_b
                )

            # ---- main loop over row tiles
            def mm(acc, qxT, kt, nq):
                nc.tensor.matmul(
                    acc, lhsT=qxT[:, kt, :], rhs=qwT[:, kt, ts(nq, 512)],
                    start=(kt == 0), stop=(kt == KT - 1),
                    skip_group_check=True,
                )

            def finish(mi, accs, amc):
                cs = small.tile([128, 1], f32, tag="small")
                nc.vector.tensor_mul(cs, amc, q_b)  # (amax/127)*meanc
                o_t = outp.tile([128, N], f16, tag="outp", name=f"o{mi}")
                for nq in range(NQ):
                    nc.scalar.activation(
                        out=o_t[:, ts(nq, 512)], in_=accs[nq],
                        func=Act.Copy, scale=cs,
                    )
                nc.scalar.dma_start(out=out_ext[ts(mi, 128), :], in_=o_t)

            if MT >= 2:
                # interleave the first two row tiles across kt so each
                # arriving qwT k-tile unlocks 7 matmuls during the W-prep ramp
                qxT0, amc0 = xq0
                qxT1, amc1 = xq1
                accs0 = [
                    pacc.tile([128, 512], f32, tag="acc", name=f"acc_0_{i}")
                    for i in range(NQ)
                ]
                accs1 = [
                    pacc.tile([128, 512], f32, tag="acc", name=f"acc_1_{i}")
                    for i in range(NQ)
                ]
                for kt in range(KT):
                    for nq in range(NQ):
                        mm(accs0[nq], qxT0, kt, nq)
                    for nq in range(NQ):
                        mm(accs1[nq], qxT1, kt, nq)
                finish(0, accs0, amc0)
                finish(1, accs1, amc1)
                start_mi = 2
            else:
                start_mi = 0

            for mi in range(start_mi, MT):
                qxT, amc = x_quant(mi)
                accs = [
                    pacc.tile([128, 512], f32, tag="acc", name=f"acc_{mi}_{i}")
                    for i in range(NQ)
                ]
                if mi == MT - 1:
                    # nq-inner: each output chunk completes as soon as its
                    # 16 accumulations are done, so the dequant + store
                    # overlap the remaining matmuls (shorter kernel tail)
                    for nq in range(NQ):
                        for kt in range(KT):
                            mm(accs[nq], qxT, kt, nq)
                else:
                    for kt in range(KT):
                        for nq in range(NQ):
                            mm(accs[nq], qxT, kt, nq)
                finish(mi, accs, amc)

    nc.compile()
    return nc


_NC_CACHE = {}


def _get_nc(rows_per_core):
    if rows_per_core not in _NC_CACHE:
        _NC_CACHE[rows_per_core] = build(rows_per_core)
    return _NC_CACHE[rows_per_core]


def run(x, weight, **spmd_kwargs):
    x = np.ascontiguousarray(np.asarray(x, dtype=np.float32))
    weight = np.asarray(weight, dtype=np.float32)
    b, s, k = x.shape
    rows = b * s
    rpc = rows // N_CORES
    xr = x.reshape(rows, k)
    wt = np.ascontiguousarray(weight.T)
    nc = _get_nc(rpc)
    in_maps = [
        {"x": xr[i * rpc : (i + 1) * rpc], "wt": wt} for i in range(N_CORES)
    ]
    res = run_bass_kernel_spmd(
        nc, in_maps, core_ids=list(range(N_CORES)), **spmd_kwargs
    )
    out = np.concatenate(
        [res.results[i]["out"] for i in range(N_CORES)], axis=0
    )
    return out.reshape(b, s, N), res


def kernel(x, weight):
    out, _ = run(x, weight)
    return out

